# revision 5
# baseline (speedup 1.0000x reference)
"""Trainium2 Bass kernel for cross-attention (single query per position, m=16 context).

Reference computation (per batch b, position n):
  q = x @ W_q                      [n, 512] -> heads h=8, d=64
  k,v = y @ W_kv                   [n, m, 512] each
  dots[h,m] = (q_h . k_mh) / 8
  attn = softmax_m(dots)
  out = (sum_m attn * v) @ W_out + b_out

Sharding: data-parallel over batch (8 batches -> 8 NeuronCores), weights replicated.
"""

import numpy as np
from contextlib import ExitStack

import concourse.bass as bass
import concourse.bacc as bacc
import concourse.mybir as mybir
import concourse.tile as tile
from concourse.bass_utils import run_bass_kernel_spmd
from concourse.masks import make_identity

B, N, M, DIM = 8, 2048, 16, 256
HEADS, DHEAD, INNER = 8, 64, 512
SCALE = DHEAD**-0.5
NCORES = 8
T = 128          # positions per tile
NT = N // T      # 16 tiles per core

F32 = mybir.dt.float32
CD = mybir.dt.bfloat16  # compute dtype for matmuls / DVE attention core


def _build_nc():
    nc = bacc.Bacc("TRN2", target_bir_lowering=False, debug=False, num_devices=NCORES)
    x = nc.dram_tensor("x", [N, DIM], F32, kind="ExternalInput").ap()
    y = nc.dram_tensor("y", [N * M, DIM], F32, kind="ExternalInput").ap()
    wq = nc.dram_tensor("wq", [DIM, INNER], F32, kind="ExternalInput").ap()
    wkv = nc.dram_tensor("wkv", [DIM, 2 * INNER], F32, kind="ExternalInput").ap()
    wout = nc.dram_tensor("wout", [INNER, DIM], F32, kind="ExternalInput").ap()
    bout = nc.dram_tensor("bout", [1, DIM], F32, kind="ExternalInput").ap()
    out = nc.dram_tensor("out", [N, DIM], F32, kind="ExternalOutput").ap()

    with tile.TileContext(nc) as tc:
        with ExitStack() as ctx:
            _body(ctx, tc, out, x, y, wq, wkv, wout, bout)
    nc.compile()
    return nc


def _body(ctx, tc, out, x, y, wq, wkv, wout, bout):
    nc = tc.nc
    consts = ctx.enter_context(tc.tile_pool(name="consts", bufs=1))
    stage = ctx.enter_context(tc.tile_pool(name="stage", bufs=2))
    ypool = ctx.enter_context(tc.tile_pool(name="ypool", bufs=2))
    work = ctx.enter_context(tc.tile_pool(name="work", bufs=2))
    tp_psum = ctx.enter_context(tc.tile_pool(name="tp_psum", bufs=2, space="PSUM"))
    k_psum = ctx.enter_context(tc.tile_pool(name="k_psum", bufs=2, space="PSUM"))
    v_psum = ctx.enter_context(tc.tile_pool(name="v_psum", bufs=2, space="PSUM"))
    o_psum = ctx.enter_context(tc.tile_pool(name="o_psum", bufs=1, space="PSUM"))

    ident = consts.tile([128, 128], F32, tag="ident")
    make_identity(nc, ident[:])
    if CD == F32:
        ident_cd = ident
    else:
        ident_cd = consts.tile([128, 128], CD, tag="ident_cd")
        nc.any.tensor_copy(ident_cd[:], ident[:])

    # --- weights: [c, cols] with contraction chunked to 128 partitions ---
    def load_w(ap, n_chunks, cols, name):
        st = consts.tile([128, n_chunks, cols], F32, tag=f"{name}_f32")
        nc.sync.dma_start(st[:], ap.rearrange("(a p) i -> p a i", p=128))
        if CD == F32:
            return st
        cd = consts.tile([128, n_chunks, cols], CD, tag=f"{name}_cd")
        nc.any.tensor_copy(cd[:], st[:])
        return cd

    wq_sb = load_w(wq, 2, INNER, "wq")
    wkv_sb = load_w(wkv, 2, 2 * INNER, "wkv")
    wout_sb = load_w(wout, 4, DIM, "wout")

    # bias: added to the out-proj psum via ones[1,128].T @ bout[1,256]
    bout_f = consts.tile([1, DIM], F32, tag="bout_f")
    nc.sync.dma_start(bout_f[:], bout)
    ones_sb = consts.tile([1, 128], CD, tag="ones")
    nc.any.memset(ones_sb[:], 1.0)
    if CD == F32:
        bout_cd = bout_f
    else:
        bout_cd = consts.tile([1, DIM], CD, tag="bout_cd")
        nc.any.tensor_copy(bout_cd[:], bout_f[:])

    x_t = x.rearrange("(t p) c -> t p c", p=T)
    y_t = y.rearrange("(t p m) c -> t p m c", p=T, m=M)
    out_t = out.rearrange("(t p) c -> t p c", p=T)

    for t in range(NT):
        # ---- load x tile, transpose to [c, pos] chunks ----
        x_sb = stage.tile([T, DIM], F32, tag="x")
        nc.sync.dma_start(x_sb[:], x_t[t])
        xT = work.tile([128, 2, 128], CD, tag="xT")
        for ci in range(2):
            ps = tp_psum.tile([128, 128], F32, tag="tp")
            nc.tensor.transpose(ps[:], x_sb[:, bass.ts(ci, 128)], ident[:])
            nc.any.tensor_copy(xT[:, ci], ps[:])

        # ---- q projection: psum[pos, 512] ----
        q_ps = o_psum.tile([T, INNER], F32, tag="q")
        for ci in range(2):
            nc.tensor.matmul(q_ps[:], xT[:, ci], wq_sb[:, ci],
                             start=(ci == 0), stop=(ci == 1))
        q_cd = work.tile([T, INNER], CD, tag="q_cd")
        nc.any.tensor_copy(q_cd[:], q_ps[:])

        # ---- load y tile [pos, m, c]; transpose all [128,128] blocks ----
        y_sb = ypool.tile([T, M, DIM], F32, tag="y")
        nc.sync.dma_start(y_sb[:], y_t[t])
        yT = ypool.tile([128, M, 2, 128], CD, tag="yT")
        for m in range(M):
            for ci in range(2):
                ps = tp_psum.tile([128, 128], F32, tag="tp")
                nc.tensor.transpose(ps[:], y_sb[:, m, bass.ts(ci, 128)], ident[:])
                nc.any.tensor_copy(yT[:, m, ci], ps[:])

        # ---- kv projection per m; evacuate k, v to SBUF in CD ----
        k_sb = work.tile([T, M, INNER], CD, tag="k")
        v_sb = work.tile([T, M, INNER], CD, tag="v")
        for m in range(M):
            k_ps = k_psum.tile([T, INNER], F32, tag="k")
            v_ps = v_psum.tile([T, INNER], F32, tag="v")
            for ci in range(2):
                nc.tensor.matmul(k_ps[:], yT[:, m, ci], wkv_sb[:, ci, 0:INNER],
                                 start=(ci == 0), stop=(ci == 1))
            for ci in range(2):
                nc.tensor.matmul(v_ps[:], yT[:, m, ci], wkv_sb[:, ci, INNER:2 * INNER],
                                 start=(ci == 0), stop=(ci == 1))
            nc.any.tensor_copy(k_sb[:, m], k_ps[:])
            nc.any.tensor_copy(v_sb[:, m], v_ps[:])

        # ---- dots[pos, m, h] = sum_d q*k ----
        dots = work.tile([T, M, HEADS], F32, tag="dots")
        for m in range(M):
            prod = work.tile([T, INNER], CD, tag="prod")
            nc.vector.tensor_mul(prod[:], q_cd[:], k_sb[:, m])
            nc.vector.tensor_reduce(
                dots[:, m], prod[:].rearrange("p (h d) -> p h d", d=DHEAD),
                axis=mybir.AxisListType.X, op=mybir.AluOpType.add)

        # ---- softmax over m (no max subtraction; |dots*SCALE| is O(5)) ----
        e_sb = work.tile([T, M, HEADS], F32, tag="e")
        nc.scalar.activation(e_sb[:], dots[:], mybir.ActivationFunctionType.Exp,
                             scale=float(SCALE))
        s_sb = work.tile([T, HEADS], F32, tag="s")
        nc.vector.tensor_reduce(s_sb[:], e_sb[:].transpose([0, 2, 1]),
                                axis=mybir.AxisListType.X, op=mybir.AluOpType.add)
        rs = work.tile([T, HEADS], F32, tag="rs")
        nc.vector.reciprocal(rs[:], s_sb[:])

        # ---- weighted sum over m: acc[pos, h, d] += e[pos,m,h] * v ----
        acc = work.tile([T, HEADS, DHEAD], F32, tag="acc")
        for m in range(M):
            prod2 = work.tile([T, HEADS, DHEAD], F32, tag="prod2")
            e_bc = e_sb[:, m].unsqueeze(2).broadcast_to([T, HEADS, DHEAD])
            nc.vector.tensor_mul(prod2[:], v_sb[:, m].rearrange("p (h d) -> p h d", d=DHEAD), e_bc)
            if m == 0:
                nc.any.tensor_copy(acc[:], prod2[:])
            else:
                nc.vector.tensor_add(acc[:], acc[:], prod2[:])

        # ---- normalize by 1/sum, cast to CD ----
        avn = work.tile([T, HEADS, DHEAD], CD, tag="avn")
        rs_bc = rs[:].unsqueeze(2).broadcast_to([T, HEADS, DHEAD])
        nc.vector.tensor_mul(avn[:], acc[:], rs_bc)

        # ---- out projection: transpose avn, matmul with W_out, add bias ----
        aoT = work.tile([128, 4, 128], CD, tag="aoT")
        avn_flat = avn[:].rearrange("p h d -> p (h d)")
        for ci in range(4):
            ps = tp_psum.tile([128, 128], CD, tag="tp")
            nc.tensor.transpose(ps[:], avn_flat[:, bass.ts(ci, 128)], ident_cd[:])
            nc.any.tensor_copy(aoT[:, ci], ps[:])

        o_ps = o_psum.tile([T, DIM], F32, tag="o")
        for ci in range(4):
            nc.tensor.matmul(o_ps[:], aoT[:, ci], wout_sb[:, ci],
                             start=(ci == 0), stop=False)
        nc.tensor.matmul(o_ps[:], ones_sb[:], bout_cd[:], start=False, stop=True)

        o_sb = stage.tile([T, DIM], F32, tag="o")
        nc.any.tensor_copy(o_sb[:], o_ps[:])
        nc.sync.dma_start(out_t[t], o_sb[:])


_NC_CACHE = {}


def get_nc():
    if "nc" not in _NC_CACHE:
        _NC_CACHE["nc"] = _build_nc()
    return _NC_CACHE["nc"]


def make_in_maps(x, y, W_q, W_kv, W_out, b_out):
    in_maps = []
    for i in range(NCORES):
        in_maps.append({
            "x": np.ascontiguousarray(x[i], dtype=np.float32),
            "y": np.ascontiguousarray(y[i], dtype=np.float32).reshape(N * M, DIM),
            "wq": np.ascontiguousarray(W_q, dtype=np.float32),
            "wkv": np.ascontiguousarray(W_kv, dtype=np.float32),
            "wout": np.ascontiguousarray(W_out, dtype=np.float32),
            "bout": np.ascontiguousarray(b_out, dtype=np.float32).reshape(1, DIM),
        })
    return in_maps


def kernel(x, y, W_q, W_kv, W_out, b_out):
    nc = get_nc()
    in_maps = make_in_maps(x, y, W_q, W_kv, W_out, b_out)
    res = run_bass_kernel_spmd(nc, in_maps, core_ids=list(range(NCORES)))
    return np.stack([res.results[i]["out"] for i in range(NCORES)]).astype(np.float32)
